# revision 30
# baseline (speedup 1.0000x reference)
"""BitLinear forward (RMSNorm + absmean ternary weight quant + absmax int8
activation quant + scaled matmul), tensor-parallel over 8 NeuronCores.

Sharding: column-parallel linear — weight rows (out_features) split 8 ways;
x is replicated; alpha (global mean |w|) via a tiny AllReduce; each core
computes y[:, shard] and the host concatenates.

Exactness: quantized activations are integers in [-127, 127] and quantized
weights are in {-1, 0, 1}, so the matmul runs in bf16 (lhsT) x fp8e4 (rhs)
with fp32 PSUM accumulation and is bit-exact (all partial sums < 2^24).

Schedule (v2): W1 weight-scan DMAs get the queue exclusively so the local
|w| sum triggers the AllReduce as early as possible; x-quant chains fill the
collective's latency window; W2 weight-quantize is pipelined across ACT+DVE
with transposes alternating between the two HWDGE rings; the matmul loop
consumes weight column-blocks in production order (ob-outer) so it starts
as soon as the first four o-tiles are quantized.
"""

import numpy as np

import concourse.bass as bass
import concourse.mybir as mybir
import concourse.tile as tile
from concourse.bass_utils import run_bass_kernel_spmd


# The walrus build available here rejects instructions carrying more than one
# attached sync-wait ("Too many sync wait commands"), which Tile emits
# routinely.  Hoist extras onto single-wait NoOps on the same engine —
# engine streams are in-order so wait-then-issue is equivalent.
MAX_ATTACHED_WAITS = 1


def _split_sync_waits(nc, max_waits=MAX_ATTACHED_WAITS):
    nhoisted = 0
    for f in nc.m.functions:
        for blk in f.blocks:
            out = []
            changed = False
            for inst in blk.instructions:
                si = inst.sync_info
                if si is not None and len(si.on_wait) > max_waits:
                    waits = list(si.on_wait)
                    for wt in waits[max_waits:]:
                        out.append(
                            mybir.InstNoOp(
                                name=f"syncsplit-{nc.next_id()}",
                                ins=[],
                                outs=[],
                                engine=inst.engine,
                                sync_info=mybir.SyncInfo(
                                    on_wait=[wt], on_update=[]
                                ),
                                bass_nofuse=True,
                            )
                        )
                        nhoisted += 1
                    inst.sync_info = mybir.SyncInfo(
                        on_wait=waits[:max_waits], on_update=list(si.on_update)
                    )
                    changed = True
                out.append(inst)
            if changed:
                blk.instructions = out
    return nhoisted


F32 = mybir.dt.float32
BF16 = mybir.dt.bfloat16
FP8 = mybir.dt.float8e4

MAGIC = 1.5 * 2.0**23  # add/sub rounds f32 to nearest int (ties to even)
EPS = 1e-6

N_CORES = 8
AFT = mybir.ActivationFunctionType
ALU = mybir.AluOpType


def build(T, K, O, n_cores):
    """One-core SPMD program: x[T,K] f32, w[O,K] f32 shard, nw[1,K] -> y[T,O]."""
    TT, KT, OT = T // 128, K // 128, O // 128
    OBN = max(1, O // 512)  # number of 512-wide output column blocks
    OBW = O // OBN
    assert OBW <= 512
    OTB = OT // OBN  # o-tiles per output block

    nc = bass.Bass(
        "TRN2", target_bir_lowering=False, debug=False, num_devices=n_cores
    )
    x = nc.dram_tensor("x", [T, K], F32, kind="ExternalInput")
    w = nc.dram_tensor("w", [O, K], F32, kind="ExternalInput")
    nw = nc.dram_tensor("nw", [1, K], F32, kind="ExternalInput")
    y = nc.dram_tensor("y", [T, O], F32, kind="ExternalOutput")

    inv_count = 1.0 / (O * n_cores * K)  # power of two for real sizes

    with tile.TileContext(nc) as tc:
        with (
            tc.tile_pool(name="const", bufs=1) as cpool,
            tc.tile_pool(name="wres", bufs=1) as wres,
            tc.tile_pool(name="big", bufs=2) as big,
            tc.tile_pool(name="stat", bufs=6) as spool,
            tc.tile_pool(name="osbp", bufs=4) as osbp,
            tc.tile_pool(name="psum", bufs=6, space="PSUM") as ps,
            tc.tile_pool(name="pss", bufs=1, space="PSUM") as pssp,
            tc.tile_pool(name="dram", bufs=1, space="DRAM") as dram,
        ):
            # ---- constants ----
            posmagic = cpool.tile([128, 1], F32, tag="posmagic")
            nc.vector.memset(posmagic[:], MAGIC)
            negmagic = cpool.tile([128, 1], F32, tag="negmagic")
            nc.vector.memset(negmagic[:], -MAGIC)
            epsb = cpool.tile([128, 1], F32, tag="epsb")
            nc.vector.memset(epsb[:], EPS)
            ones_col = cpool.tile([128, 1], F32, tag="ones_col")
            nc.vector.memset(ones_col[:], 1.0)
            ones_row = cpool.tile([1, 128], F32, tag="ones_row")
            nc.vector.memset(ones_row[:], 1.0)
            alpha_bc = cpool.tile([128, 1], F32, tag="alpha_bc")
            inv_alpha_bc = cpool.tile([128, 1], F32, tag="inv_alpha_bc")
            nw_rep = cpool.tile([128, K], BF16, tag="nw_rep")
            wsum = cpool.tile([128, OT], F32, tag="wsum")

            # resident transposed ternary weights, fp8 (exact for -1/0/1)
            # ot-major layout: [128, OT*KT*128]; o-tile ot owns the contiguous
            # column range [ot*K, (ot+1)*K), kt-subblocks of 128 inside it
            wqT = wres.tile([128, OT * K], FP8, tag="wqT")
            wqT_r = wqT[:].rearrange("p (ot kt f) -> p ot kt f", kt=KT, f=128)

            # replicate norm_weight to all 128 partitions BEFORE the W1 loads
            # claim the SWDGE FIFO: the first hop casts f32->bf16 (SWDGE-only,
            # 16 KB, ~2us), the doubling hops ride the Scalar HWDGE ring so
            # the x-quant chains are not gated on the whole W1 phase.
            nc.gpsimd.dma_start(nw_rep[0:1, :], nw.ap())
            p = 1
            while p < 128:
                nc.scalar.dma_start(nw_rep[p : 2 * p, :], nw_rep[0:p, :])
                p *= 2

            # ---- phase W1: per-shard |w| row sums (queue-exclusive DMAs) ----
            for ot in range(OT):
                wt = big.tile([128, K], F32, tag="bf32a", name=f"wt_{ot}")
                nc.gpsimd.dma_start(wt[:], w[ot * 128 : (ot + 1) * 128, :])
                absw = big.tile([128, K], BF16, tag="s16a", name=f"absw_{ot}")
                nc.scalar.activation(
                    absw[:], wt[:], AFT.Abs, accum_out=wsum[:, ot : ot + 1]
                )

            # ---- x quant chains (fill the collective's latency window) ----
            # x loads ride the Sync HWDGE ring so they are not head-of-line
            # blocked behind the SWDGE weight-load FIFO or the collective.
            sys_ = {}

            def quant_chain(tt):
                xin = big.tile([128, K], F32, tag="bf32a", name=f"xin_{tt}")
                nc.sync.dma_start(xin[:], x[tt * 128 : (tt + 1) * 128, :])

                x2 = big.tile([128, K], BF16, tag="s16a", name=f"x2_{tt}")
                ss = spool.tile([128, 1], F32, tag="ss", name=f"ss_{tt}")
                nc.scalar.activation(x2[:], xin[:], AFT.Square, accum_out=ss[:])

                u = big.tile([128, K], F32, tag="bf32b", name=f"u_{tt}")
                nc.vector.tensor_mul(u[:], xin[:], nw_rep[:])
                graw = spool.tile([128, 1], F32, tag="graw", name=f"graw_{tt}")
                nc.vector.tensor_reduce(
                    graw[:],
                    u[:],
                    axis=mybir.AxisListType.X,
                    op=ALU.max,
                    apply_absolute_value=True,
                )
                g = spool.tile([128, 1], F32, tag="g", name=f"g_{tt}")
                nc.vector.tensor_scalar_max(g[:], graw[:], 1e-10)

                invg = spool.tile([128, 1], F32, tag="invg", name=f"invg_{tt}")
                nc.vector.reciprocal(invg[:], g[:])
                s127 = spool.tile([128, 1], F32, tag="s127", name=f"s127_{tt}")
                nc.vector.tensor_scalar_mul(s127[:], invg[:], 127.0)
                rms = spool.tile([128, 1], F32, tag="rms", name=f"rms_{tt}")
                nc.scalar.activation(
                    rms[:], ss[:], AFT.Sqrt, bias=epsb[:], scale=1.0 / K
                )
                invrms = spool.tile([128, 1], F32, tag="invrms", name=f"invrms_{tt}")
                nc.vector.reciprocal(invrms[:], rms[:])
                gor = spool.tile([128, 1], F32, tag="gor", name=f"gor_{tt}")
                nc.vector.tensor_mul(gor[:], g[:], invrms[:])
                sys_[tt] = gor

                # round(u * 127/g) via magic add/sub; mul+add on ACT, sub on DVE
                q1 = big.tile([128, K], F32, tag="bf32b", name=f"q1_{tt}")
                nc.scalar.activation(
                    q1[:], u[:], AFT.Identity, bias=posmagic[:], scale=s127[:]
                )
                xq = big.tile([128, K], BF16, tag="s16a", name=f"xq_{tt}")
                nc.vector.tensor_scalar_add(xq[:], q1[:], -MAGIC)

                # transpose all KT 128x128 blocks in one DMA-transpose call.
                # DMA_TRANSPOSE occupies the issuing engine for the transfer
                # duration. Prefetch-chain transposes ride the idle Sync
                # engine (before W2 claims it); in-loop ones ride Scalar,
                # which has slack during the matmul loop — this keeps them
                # out of the Sync FIFO behind W2's 16 transposes.
                xqT = big.tile([128, K], BF16, tag="xqT", name=f"xqT_{tt}", bufs=3)
                eng = nc.sync if tt < 3 else nc.scalar
                eng.dma_start(
                    xqT[:].rearrange("p (j f) -> p j f", f=128),
                    xq[:].rearrange("p (j f) -> p j f", f=128),
                    transpose=True,
                )
                return xqT

            xqTs = {}
            NPRE = 3
            for tt in range(min(NPRE, TT)):
                xqTs[tt] = quant_chain(tt)

            # ---- alpha: local reduce -> AllReduce -> matmul broadcast ----
            # the DRAM round-trip DMAs ride the Scalar HWDGE ring: the
            # result readback waits on the collective, and on the SWDGE
            # FIFO it would head-of-line block the W2 weight loads.
            wred = spool.tile([128, 1], F32, tag="wred")
            nc.vector.reduce_sum(wred[:], wsum[:], axis=mybir.AxisListType.X)
            pss = pssp.tile([1, 1], F32, tag="pss", name="pss")
            nc.tensor.matmul(pss[:], wred[:], ones_col[:], start=True, stop=True)
            total_sb = spool.tile([1, 8], F32, tag="total_sb")
            nc.vector.memset(total_sb[:], 0.0)
            nc.vector.tensor_copy(total_sb[:, 0:1], pss[:])

            cc_in = dram.tile([1, 8], F32, tag="cc_in")
            cc_out = dram.tile([1, 8], F32, tag="cc_out")
            nc.scalar.dma_start(cc_in[:], total_sb[:])
            nc.gpsimd.collective_compute(
                "AllReduce",
                ALU.add,
                replica_groups=[list(range(n_cores))],
                ins=[cc_in.opt()],
                outs=[cc_out.opt()],
            )
            gtot = spool.tile([1, 1], F32, tag="gtot")
            nc.scalar.dma_start(gtot[:], cc_out[:, 0:1])
            alpha_s = spool.tile([1, 1], F32, tag="alpha_s")
            nc.vector.tensor_scalar(
                out=alpha_s[:],
                in0=gtot[:],
                scalar1=inv_count,
                scalar2=1e-10,
                op0=ALU.mult,
                op1=ALU.max,
            )
            # broadcast alpha to 128 partitions with one tiny PE matmul
            psb = pssp.tile([128, 1], F32, tag="psb", name="psb")
            nc.tensor.matmul(psb[:], ones_row[:], alpha_s[:], start=True, stop=True)
            nc.scalar.copy(alpha_bc[:], psb[:])
            nc.vector.reciprocal(inv_alpha_bc[:], alpha_bc[:])

            # ---- phase W2: quantize + transpose weights ----
            # round(w/alpha) via magic add/sub stays UNCLIPPED in bf16 (small
            # ints are exact); the clip to [-1,1] fuses into the post-
            # transpose fp8 convert as one dual-op tensor_scalar on DVE.
            # Engine streams are strict in-order, so a single chain paces at
            # the cross-engine ping-pong rate: run even o-tiles entirely on
            # ACT and odd o-tiles entirely on DVE as two independent
            # pipelines (identical fp32 scale*x+bias arithmetic on both).
            for ot in range(OT):
                # split each 2MB load into halves on two DMA paths so the
                # landing latency (~9.6us whole) halves and overlaps compute
                wt2 = big.tile([128, K], F32, tag="bf32a", name=f"wt2_{ot}")
                nc.gpsimd.dma_start(
                    wt2[:, 0 : K // 2], w[ot * 128 : (ot + 1) * 128, 0 : K // 2]
                )
                nc.sync.dma_start(
                    wt2[:, K // 2 : K], w[ot * 128 : (ot + 1) * 128, K // 2 : K]
                )
                # round(w/alpha) via the exact f32 magic (the bf16-magic
                # one-op variant double-rounds: f32 ulp at 192 is 2^-16, so
                # ~1e-5 of weights flip across the +-0.5 thresholds - too
                # many for the error gate). Even o-tiles run on ACT, odd on
                # DVE, as two independent pipelines.
                wdiv = big.tile([128, K], F32, tag="bf32b", name=f"wdiv_{ot}")
                rnd = big.tile([128, K], BF16, tag="s16a", name=f"rnd_{ot}")
                if ot % 2 == 0:
                    nc.scalar.activation(
                        wdiv[:],
                        wt2[:],
                        AFT.Identity,
                        bias=posmagic[:],
                        scale=inv_alpha_bc[:],
                    )
                    nc.scalar.activation(
                        rnd[:], wdiv[:], AFT.Identity, bias=negmagic[:], scale=1.0
                    )
                else:
                    nc.vector.tensor_scalar(
                        out=wdiv[:],
                        in0=wt2[:],
                        scalar1=inv_alpha_bc[:],
                        scalar2=MAGIC,
                        op0=ALU.mult,
                        op1=ALU.add,
                    )
                    nc.vector.tensor_scalar(
                        out=rnd[:],
                        in0=wdiv[:],
                        scalar1=MAGIC,
                        scalar2=-1.0,
                        op0=ALU.subtract,
                        op1=ALU.max,
                    )
                # transpose on the Sync engine (engine-blocking op; Sync is idle)
                rndT = big.tile([128, K], BF16, tag="s16b", name=f"rndT_{ot}")
                nc.sync.dma_start(
                    rndT[:].rearrange("p (j f) -> p j f", f=128),
                    rnd[:].rearrange("p (j f) -> p j f", f=128),
                    transpose=True,
                )
                # fused clip to [-1,1] + convert bf16 -> fp8 on GPSIMD: the
                # third compute engine is idle here, and moving this stage
                # out of the ACT/DVE streams removes its hoisted sync-wait
                # NOPs from the quantize pipelines' strict in-order streams.
                nc.gpsimd.tensor_scalar(
                    out=wqT[:, ot * K : (ot + 1) * K],
                    in0=rndT[:],
                    scalar1=-1.0,
                    scalar2=1.0,
                    op0=ALU.max,
                    op1=ALU.min,
                )

            # ---- main loop: ob-outer matmuls track the W2 production wave ---
            for tt in range(TT):
                if tt + NPRE < TT:
                    xqTs[tt + NPRE] = quant_chain(tt + NPRE)
                xqT = xqTs.pop(tt)
                gor = sys_.pop(tt)
                sy = spool.tile([128, 1], F32, tag="sy", name=f"sy_{tt}")
                nc.vector.tensor_scalar(
                    out=sy[:],
                    in0=gor[:],
                    scalar1=alpha_bc[:],
                    scalar2=1.0 / 127.0,
                    op0=ALU.mult,
                    op1=ALU.mult,
                )

                # rotated ob start for the first OBN row-tiles: tt consumes
                # weight blocks in the order W2 produces them (tt=0 follows
                # the wave from ob0, tt=1 picks up at ob1, ...), absorbing
                # matmul work into the W2 window while the x-chains stay
                # gated behind W2's buffer ring, so W2 keeps ACT/DVE.
                rot = min(tt, OBN - 1)
                for ob in [(o + rot) % OBN for o in range(OBN)]:
                    psum = ps.tile([128, OBW], F32, tag="ps", name=f"ps_{tt}_{ob}")
                    for kt in range(KT):
                        nc.tensor.matmul(
                            psum[:],
                            xqT[:, kt * 128 : (kt + 1) * 128],
                            wqT_r[:, ob * OTB : (ob + 1) * OTB, kt, :],
                            start=(kt == 0),
                            stop=(kt == KT - 1),
                        )
                    # epilogue on ACT: scale by alpha*gamma/127, store this slice
                    osb = osbp.tile(
                        [128, OBW], F32, tag="osb", name=f"osb_{tt}_{ob}"
                    )
                    nc.scalar.mul(osb[:], psum[:], sy[:])
                    nc.gpsimd.dma_start(
                        y[tt * 128 : (tt + 1) * 128, ob * OBW : (ob + 1) * OBW],
                        osb[:],
                    )

    return nc


_nc_cache = {}


def _get_nc(T, K, O, n_cores):
    key = (T, K, O, n_cores)
    if key not in _nc_cache:
        nc = build(T, K, O, n_cores)
        _split_sync_waits(nc)  # HW-only fixup; CoreSim rejects bare NoOps
        _nc_cache[key] = nc
    return _nc_cache[key]


def kernel(x: np.ndarray, weight: np.ndarray, norm_weight: np.ndarray) -> np.ndarray:
    B, S, K = x.shape
    T = B * S
    Ofull, _ = weight.shape
    O = Ofull // N_CORES

    nc = _get_nc(T, K, O, N_CORES)

    xf = np.ascontiguousarray(x.reshape(T, K).astype(np.float32, copy=False))
    nwf = np.ascontiguousarray(norm_weight.reshape(1, K).astype(np.float32, copy=False))
    in_maps = [
        {
            "x": xf,
            "w": np.ascontiguousarray(weight[i * O : (i + 1) * O]),
            "nw": nwf,
        }
        for i in range(N_CORES)
    ]
    res = run_bass_kernel_spmd(nc, in_maps, list(range(N_CORES))).results
    y = np.concatenate([res[i]["y"] for i in range(N_CORES)], axis=1)
    return y.reshape(B, S, Ofull)


# revision 31
# speedup vs baseline: 1.7320x; 1.7320x over previous
"""BitLinear forward (RMSNorm + absmean ternary weight quant + absmax int8
activation quant + scaled matmul), tensor-parallel over 8 NeuronCores.

Sharding: column-parallel linear — weight rows (out_features) split 8 ways;
x is replicated; alpha (global mean |w|) via a tiny AllReduce; each core
computes y[:, shard] and the host concatenates.

Exactness: quantized activations are integers in [-127, 127] and quantized
weights are in {-1, 0, 1}, so the matmul runs in bf16 (lhsT) x fp8e4 (rhs)
with fp32 PSUM accumulation and is bit-exact (all partial sums < 2^24).

Schedule (v2): W1 weight-scan DMAs get the queue exclusively so the local
|w| sum triggers the AllReduce as early as possible; x-quant chains fill the
collective's latency window; W2 weight-quantize is pipelined across ACT+DVE
with transposes alternating between the two HWDGE rings; the matmul loop
consumes weight column-blocks in production order (ob-outer) so it starts
as soon as the first four o-tiles are quantized.
"""

import numpy as np

import concourse.bass as bass
import concourse.mybir as mybir
import concourse.tile as tile
from concourse.bass_utils import run_bass_kernel_spmd


# The walrus build available here rejects instructions carrying more than one
# attached sync-wait ("Too many sync wait commands"), which Tile emits
# routinely.  Hoist extras onto single-wait NoOps on the same engine —
# engine streams are in-order so wait-then-issue is equivalent.
MAX_ATTACHED_WAITS = 1


def _split_sync_waits(nc, max_waits=MAX_ATTACHED_WAITS):
    nhoisted = 0
    for f in nc.m.functions:
        for blk in f.blocks:
            out = []
            changed = False
            for inst in blk.instructions:
                si = inst.sync_info
                if si is not None and len(si.on_wait) > max_waits:
                    waits = list(si.on_wait)
                    for wt in waits[max_waits:]:
                        out.append(
                            mybir.InstNoOp(
                                name=f"syncsplit-{nc.next_id()}",
                                ins=[],
                                outs=[],
                                engine=inst.engine,
                                sync_info=mybir.SyncInfo(
                                    on_wait=[wt], on_update=[]
                                ),
                                bass_nofuse=True,
                            )
                        )
                        nhoisted += 1
                    inst.sync_info = mybir.SyncInfo(
                        on_wait=waits[:max_waits], on_update=list(si.on_update)
                    )
                    changed = True
                out.append(inst)
            if changed:
                blk.instructions = out
    return nhoisted


F32 = mybir.dt.float32
BF16 = mybir.dt.bfloat16
FP8 = mybir.dt.float8e4

MAGIC = 1.5 * 2.0**23  # add/sub rounds f32 to nearest int (ties to even)
EPS = 1e-6

N_CORES = 8
AFT = mybir.ActivationFunctionType
ALU = mybir.AluOpType


def build(T, K, O, n_cores):
    """One-core SPMD program: x[T,K] f32, w[O,K] f32 shard, nw[1,K] -> y[T,O]."""
    TT, KT, OT = T // 128, K // 128, O // 128
    OBN = max(1, O // 512)  # number of 512-wide output column blocks
    OBW = O // OBN
    assert OBW <= 512
    OTB = OT // OBN  # o-tiles per output block

    nc = bass.Bass(
        "TRN2", target_bir_lowering=False, debug=False, num_devices=n_cores
    )
    x = nc.dram_tensor("x", [T, K], F32, kind="ExternalInput")
    w = nc.dram_tensor("w", [O, K], F32, kind="ExternalInput")
    nw = nc.dram_tensor("nw", [1, K], F32, kind="ExternalInput")
    y = nc.dram_tensor("y", [T, O], F32, kind="ExternalOutput")

    inv_count = 1.0 / (O * n_cores * K)  # power of two for real sizes

    with tile.TileContext(nc) as tc:
        with (
            tc.tile_pool(name="const", bufs=1) as cpool,
            tc.tile_pool(name="wres", bufs=1) as wres,
            tc.tile_pool(name="big", bufs=2) as big,
            tc.tile_pool(name="stat", bufs=6) as spool,
            tc.tile_pool(name="osbp", bufs=4) as osbp,
            tc.tile_pool(name="psum", bufs=6, space="PSUM") as ps,
            tc.tile_pool(name="pss", bufs=1, space="PSUM") as pssp,
            tc.tile_pool(name="dram", bufs=1, space="DRAM") as dram,
        ):
            # ---- constants ----
            posmagic = cpool.tile([128, 1], F32, tag="posmagic")
            nc.vector.memset(posmagic[:], MAGIC)
            negmagic = cpool.tile([128, 1], F32, tag="negmagic")
            nc.vector.memset(negmagic[:], -MAGIC)
            epsb = cpool.tile([128, 1], F32, tag="epsb")
            nc.vector.memset(epsb[:], EPS)
            ones_col = cpool.tile([128, 1], F32, tag="ones_col")
            nc.vector.memset(ones_col[:], 1.0)
            ones_row = cpool.tile([1, 128], F32, tag="ones_row")
            nc.vector.memset(ones_row[:], 1.0)
            alpha_bc = cpool.tile([128, 1], F32, tag="alpha_bc")
            inv_alpha_bc = cpool.tile([128, 1], F32, tag="inv_alpha_bc")
            nw_rep = cpool.tile([128, K], BF16, tag="nw_rep")
            wsum = cpool.tile([128, OT], F32, tag="wsum")

            # resident transposed ternary weights, fp8 (exact for -1/0/1)
            # ot-major layout: [128, OT*KT*128]; o-tile ot owns the contiguous
            # column range [ot*K, (ot+1)*K), kt-subblocks of 128 inside it
            wqT = wres.tile([128, OT * K], FP8, tag="wqT")
            wqT_r = wqT[:].rearrange("p (ot kt f) -> p ot kt f", kt=KT, f=128)

            # replicate norm_weight to all 128 partitions BEFORE the W1 loads
            # claim the SWDGE FIFO: the first hop casts f32->bf16 (SWDGE-only,
            # 16 KB, ~2us), the doubling hops ride the Scalar HWDGE ring so
            # the x-quant chains are not gated on the whole W1 phase.
            nc.gpsimd.dma_start(nw_rep[0:1, :], nw.ap())
            p = 1
            while p < 128:
                nc.scalar.dma_start(nw_rep[p : 2 * p, :], nw_rep[0:p, :])
                p *= 2

            # ---- phase W1: per-shard |w| row sums (queue-exclusive DMAs) ----
            for ot in range(OT):
                wt = big.tile([128, K], F32, tag="bf32a", name=f"wt_{ot}")
                nc.gpsimd.dma_start(wt[:], w[ot * 128 : (ot + 1) * 128, :])
                absw = big.tile([128, K], BF16, tag="s16a", name=f"absw_{ot}")
                nc.scalar.activation(
                    absw[:], wt[:], AFT.Abs, accum_out=wsum[:, ot : ot + 1]
                )

            # ---- x quant chains (fill the collective's latency window) ----
            # x loads ride the Sync HWDGE ring so they are not head-of-line
            # blocked behind the SWDGE weight-load FIFO or the collective.
            sys_ = {}

            def quant_chain(tt):
                xin = big.tile([128, K], F32, tag="bf32a", name=f"xin_{tt}")
                nc.sync.dma_start(xin[:], x[tt * 128 : (tt + 1) * 128, :])

                x2 = big.tile([128, K], BF16, tag="s16a", name=f"x2_{tt}")
                ss = spool.tile([128, 1], F32, tag="ss", name=f"ss_{tt}")
                nc.scalar.activation(x2[:], xin[:], AFT.Square, accum_out=ss[:])

                u = big.tile([128, K], F32, tag="bf32b", name=f"u_{tt}")
                nc.vector.tensor_mul(u[:], xin[:], nw_rep[:])
                graw = spool.tile([128, 1], F32, tag="graw", name=f"graw_{tt}")
                nc.vector.tensor_reduce(
                    graw[:],
                    u[:],
                    axis=mybir.AxisListType.X,
                    op=ALU.max,
                    apply_absolute_value=True,
                )
                g = spool.tile([128, 1], F32, tag="g", name=f"g_{tt}")
                nc.vector.tensor_scalar_max(g[:], graw[:], 1e-10)

                invg = spool.tile([128, 1], F32, tag="invg", name=f"invg_{tt}")
                nc.vector.reciprocal(invg[:], g[:])
                s127 = spool.tile([128, 1], F32, tag="s127", name=f"s127_{tt}")
                nc.vector.tensor_scalar_mul(s127[:], invg[:], 127.0)
                rms = spool.tile([128, 1], F32, tag="rms", name=f"rms_{tt}")
                nc.scalar.activation(
                    rms[:], ss[:], AFT.Sqrt, bias=epsb[:], scale=1.0 / K
                )
                invrms = spool.tile([128, 1], F32, tag="invrms", name=f"invrms_{tt}")
                nc.vector.reciprocal(invrms[:], rms[:])
                gor = spool.tile([128, 1], F32, tag="gor", name=f"gor_{tt}")
                nc.vector.tensor_mul(gor[:], g[:], invrms[:])
                sys_[tt] = gor

                # round(u * 127/g) via magic add/sub; mul+add on ACT, sub on DVE
                q1 = big.tile([128, K], F32, tag="bf32b", name=f"q1_{tt}")
                nc.scalar.activation(
                    q1[:], u[:], AFT.Identity, bias=posmagic[:], scale=s127[:]
                )
                xq = big.tile([128, K], BF16, tag="s16a", name=f"xq_{tt}")
                nc.vector.tensor_scalar_add(xq[:], q1[:], -MAGIC)

                # transpose all KT 128x128 blocks in one DMA-transpose call.
                # DMA_TRANSPOSE occupies the issuing engine for the transfer
                # duration. Prefetch-chain transposes ride the idle Sync
                # engine (before W2 claims it); in-loop ones ride Scalar,
                # which has slack during the matmul loop — this keeps them
                # out of the Sync FIFO behind W2's 16 transposes.
                xqT = big.tile([128, K], BF16, tag="xqT", name=f"xqT_{tt}", bufs=3)
                eng = nc.sync if tt < 3 else nc.scalar
                eng.dma_start(
                    xqT[:].rearrange("p (j f) -> p j f", f=128),
                    xq[:].rearrange("p (j f) -> p j f", f=128),
                    transpose=True,
                )
                return xqT

            xqTs = {}
            NPRE = 3
            for tt in range(min(NPRE, TT)):
                xqTs[tt] = quant_chain(tt)

            # ---- alpha: local reduce -> AllReduce -> matmul broadcast ----
            # the DRAM round-trip DMAs ride the Scalar HWDGE ring: the
            # result readback waits on the collective, and on the SWDGE
            # FIFO it would head-of-line block the W2 weight loads.
            wred = spool.tile([128, 1], F32, tag="wred")
            nc.vector.reduce_sum(wred[:], wsum[:], axis=mybir.AxisListType.X)
            pss = pssp.tile([1, 1], F32, tag="pss", name="pss")
            nc.tensor.matmul(pss[:], wred[:], ones_col[:], start=True, stop=True)
            total_sb = spool.tile([1, 8], F32, tag="total_sb")
            nc.vector.memset(total_sb[:], 0.0)
            nc.vector.tensor_copy(total_sb[:, 0:1], pss[:])

            cc_in = dram.tile([1, 8], F32, tag="cc_in")
            cc_out = dram.tile([1, 8], F32, tag="cc_out")
            nc.scalar.dma_start(cc_in[:], total_sb[:])
            nc.gpsimd.collective_compute(
                "AllReduce",
                ALU.add,
                replica_groups=[list(range(n_cores))],
                ins=[cc_in.opt()],
                outs=[cc_out.opt()],
            )
            gtot = spool.tile([1, 1], F32, tag="gtot")
            nc.scalar.dma_start(gtot[:], cc_out[:, 0:1])
            alpha_s = spool.tile([1, 1], F32, tag="alpha_s")
            nc.vector.tensor_scalar(
                out=alpha_s[:],
                in0=gtot[:],
                scalar1=inv_count,
                scalar2=1e-10,
                op0=ALU.mult,
                op1=ALU.max,
            )
            # broadcast alpha to 128 partitions with one tiny PE matmul
            psb = pssp.tile([128, 1], F32, tag="psb", name="psb")
            nc.tensor.matmul(psb[:], ones_row[:], alpha_s[:], start=True, stop=True)
            nc.scalar.copy(alpha_bc[:], psb[:])
            nc.vector.reciprocal(inv_alpha_bc[:], alpha_bc[:])

            # ---- phase W2: quantize + transpose weights ----
            # round(w/alpha) via magic add/sub stays UNCLIPPED in bf16 (small
            # ints are exact); the clip to [-1,1] fuses into the post-
            # transpose fp8 convert as one dual-op tensor_scalar on DVE.
            # Engine streams are strict in-order, so a single chain paces at
            # the cross-engine ping-pong rate: run even o-tiles entirely on
            # ACT and odd o-tiles entirely on DVE as two independent
            # pipelines (identical fp32 scale*x+bias arithmetic on both).
            for ot in range(OT):
                # split each 2MB load into halves on two DMA paths so the
                # landing latency (~9.6us whole) halves and overlaps compute
                wt2 = big.tile([128, K], F32, tag="bf32a", name=f"wt2_{ot}")
                nc.gpsimd.dma_start(
                    wt2[:, 0 : K // 2], w[ot * 128 : (ot + 1) * 128, 0 : K // 2]
                )
                nc.sync.dma_start(
                    wt2[:, K // 2 : K], w[ot * 128 : (ot + 1) * 128, K // 2 : K]
                )
                # round(w/alpha) via the exact f32 magic (the bf16-magic
                # one-op variant double-rounds: f32 ulp at 192 is 2^-16, so
                # ~1e-5 of weights flip across the +-0.5 thresholds - too
                # many for the error gate). Even o-tiles run on ACT, odd on
                # DVE, as two independent pipelines.
                wdiv = big.tile([128, K], F32, tag="bf32b", name=f"wdiv_{ot}")
                rnd = big.tile([128, K], BF16, tag="s16a", name=f"rnd_{ot}")
                if ot % 2 == 0:
                    nc.scalar.activation(
                        wdiv[:],
                        wt2[:],
                        AFT.Identity,
                        bias=posmagic[:],
                        scale=inv_alpha_bc[:],
                    )
                    nc.scalar.activation(
                        rnd[:], wdiv[:], AFT.Identity, bias=negmagic[:], scale=1.0
                    )
                else:
                    nc.vector.tensor_scalar(
                        out=wdiv[:],
                        in0=wt2[:],
                        scalar1=inv_alpha_bc[:],
                        scalar2=MAGIC,
                        op0=ALU.mult,
                        op1=ALU.add,
                    )
                    nc.vector.tensor_scalar(
                        out=rnd[:],
                        in0=wdiv[:],
                        scalar1=MAGIC,
                        scalar2=-1.0,
                        op0=ALU.subtract,
                        op1=ALU.max,
                    )
                # transpose on the Sync engine (engine-blocking op; Sync is idle)
                rndT = big.tile([128, K], BF16, tag="s16b", name=f"rndT_{ot}")
                nc.sync.dma_start(
                    rndT[:].rearrange("p (j f) -> p j f", f=128),
                    rnd[:].rearrange("p (j f) -> p j f", f=128),
                    transpose=True,
                )
                # fused clip to [-1,1] + convert bf16 -> fp8 on DVE
                # (GPSIMD computes this correctly but ~15x slower: moving it
                # there cost +1ms total - Q7 elementwise is not competitive)
                nc.vector.tensor_scalar(
                    out=wqT[:, ot * K : (ot + 1) * K],
                    in0=rndT[:],
                    scalar1=-1.0,
                    scalar2=1.0,
                    op0=ALU.max,
                    op1=ALU.min,
                )

            # ---- main loop: ob-outer matmuls track the W2 production wave ---
            for tt in range(TT):
                if tt + NPRE < TT:
                    xqTs[tt + NPRE] = quant_chain(tt + NPRE)
                xqT = xqTs.pop(tt)
                gor = sys_.pop(tt)
                sy = spool.tile([128, 1], F32, tag="sy", name=f"sy_{tt}")
                nc.vector.tensor_scalar(
                    out=sy[:],
                    in0=gor[:],
                    scalar1=alpha_bc[:],
                    scalar2=1.0 / 127.0,
                    op0=ALU.mult,
                    op1=ALU.mult,
                )

                # rotated ob start for the first OBN row-tiles: tt consumes
                # weight blocks in the order W2 produces them (tt=0 follows
                # the wave from ob0, tt=1 picks up at ob1, ...), absorbing
                # matmul work into the W2 window while the x-chains stay
                # gated behind W2's buffer ring, so W2 keeps ACT/DVE.
                rot = min(tt, OBN - 1)
                for ob in [(o + rot) % OBN for o in range(OBN)]:
                    psum = ps.tile([128, OBW], F32, tag="ps", name=f"ps_{tt}_{ob}")
                    for kt in range(KT):
                        nc.tensor.matmul(
                            psum[:],
                            xqT[:, kt * 128 : (kt + 1) * 128],
                            wqT_r[:, ob * OTB : (ob + 1) * OTB, kt, :],
                            start=(kt == 0),
                            stop=(kt == KT - 1),
                        )
                    # epilogue on ACT: scale by alpha*gamma/127, store this slice
                    osb = osbp.tile(
                        [128, OBW], F32, tag="osb", name=f"osb_{tt}_{ob}"
                    )
                    nc.scalar.mul(osb[:], psum[:], sy[:])
                    nc.gpsimd.dma_start(
                        y[tt * 128 : (tt + 1) * 128, ob * OBW : (ob + 1) * OBW],
                        osb[:],
                    )

    return nc


_nc_cache = {}


def _get_nc(T, K, O, n_cores):
    key = (T, K, O, n_cores)
    if key not in _nc_cache:
        nc = build(T, K, O, n_cores)
        _split_sync_waits(nc)  # HW-only fixup; CoreSim rejects bare NoOps
        _nc_cache[key] = nc
    return _nc_cache[key]


def kernel(x: np.ndarray, weight: np.ndarray, norm_weight: np.ndarray) -> np.ndarray:
    B, S, K = x.shape
    T = B * S
    Ofull, _ = weight.shape
    O = Ofull // N_CORES

    nc = _get_nc(T, K, O, N_CORES)

    xf = np.ascontiguousarray(x.reshape(T, K).astype(np.float32, copy=False))
    nwf = np.ascontiguousarray(norm_weight.reshape(1, K).astype(np.float32, copy=False))
    in_maps = [
        {
            "x": xf,
            "w": np.ascontiguousarray(weight[i * O : (i + 1) * O]),
            "nw": nwf,
        }
        for i in range(N_CORES)
    ]
    res = run_bass_kernel_spmd(nc, in_maps, list(range(N_CORES))).results
    y = np.concatenate([res[i]["y"] for i in range(N_CORES)], axis=1)
    return y.reshape(B, S, Ofull)


# revision 32
# speedup vs baseline: 1.7346x; 1.0015x over previous
"""BitLinear forward (RMSNorm + absmean ternary weight quant + absmax int8
activation quant + scaled matmul), tensor-parallel over 8 NeuronCores.

Sharding: column-parallel linear — weight rows (out_features) split 8 ways;
x is replicated; alpha (global mean |w|) via a tiny AllReduce; each core
computes y[:, shard] and the host concatenates.

Exactness: quantized activations are integers in [-127, 127] and quantized
weights are in {-1, 0, 1}, so the matmul runs in bf16 (lhsT) x fp8e4 (rhs)
with fp32 PSUM accumulation and is bit-exact (all partial sums < 2^24).

Schedule (v2): W1 weight-scan DMAs get the queue exclusively so the local
|w| sum triggers the AllReduce as early as possible; x-quant chains fill the
collective's latency window; W2 weight-quantize is pipelined across ACT+DVE
with transposes alternating between the two HWDGE rings; the matmul loop
consumes weight column-blocks in production order (ob-outer) so it starts
as soon as the first four o-tiles are quantized.
"""

import numpy as np

import concourse.bass as bass
import concourse.mybir as mybir
import concourse.tile as tile
from concourse.bass_utils import run_bass_kernel_spmd


# The walrus build available here rejects instructions carrying more than one
# attached sync-wait ("Too many sync wait commands"), which Tile emits
# routinely.  Hoist extras onto single-wait NoOps on the same engine —
# engine streams are in-order so wait-then-issue is equivalent.
MAX_ATTACHED_WAITS = 1


def _split_sync_waits(nc, max_waits=MAX_ATTACHED_WAITS):
    nhoisted = 0
    for f in nc.m.functions:
        for blk in f.blocks:
            out = []
            changed = False
            for inst in blk.instructions:
                si = inst.sync_info
                if si is not None and len(si.on_wait) > max_waits:
                    waits = list(si.on_wait)
                    for wt in waits[max_waits:]:
                        out.append(
                            mybir.InstNoOp(
                                name=f"syncsplit-{nc.next_id()}",
                                ins=[],
                                outs=[],
                                engine=inst.engine,
                                sync_info=mybir.SyncInfo(
                                    on_wait=[wt], on_update=[]
                                ),
                                bass_nofuse=True,
                            )
                        )
                        nhoisted += 1
                    inst.sync_info = mybir.SyncInfo(
                        on_wait=waits[:max_waits], on_update=list(si.on_update)
                    )
                    changed = True
                out.append(inst)
            if changed:
                blk.instructions = out
    return nhoisted


F32 = mybir.dt.float32
BF16 = mybir.dt.bfloat16
FP8 = mybir.dt.float8e4

MAGIC = 1.5 * 2.0**23  # add/sub rounds f32 to nearest int (ties to even)
EPS = 1e-6

N_CORES = 8
AFT = mybir.ActivationFunctionType
ALU = mybir.AluOpType


def build(T, K, O, n_cores):
    """One-core SPMD program: x[T,K] f32, w[O,K] f32 shard, nw[1,K] -> y[T,O]."""
    TT, KT, OT = T // 128, K // 128, O // 128
    OBN = max(1, O // 512)  # number of 512-wide output column blocks
    OBW = O // OBN
    assert OBW <= 512
    OTB = OT // OBN  # o-tiles per output block

    nc = bass.Bass(
        "TRN2", target_bir_lowering=False, debug=False, num_devices=n_cores
    )
    x = nc.dram_tensor("x", [T, K], F32, kind="ExternalInput")
    w = nc.dram_tensor("w", [O, K], F32, kind="ExternalInput")
    nw = nc.dram_tensor("nw", [1, K], F32, kind="ExternalInput")
    y = nc.dram_tensor("y", [T, O], F32, kind="ExternalOutput")

    inv_count = 1.0 / (O * n_cores * K)  # power of two for real sizes

    with tile.TileContext(nc) as tc:
        with (
            tc.tile_pool(name="const", bufs=1) as cpool,
            tc.tile_pool(name="wres", bufs=1) as wres,
            tc.tile_pool(name="big", bufs=2) as big,
            tc.tile_pool(name="stat", bufs=6) as spool,
            tc.tile_pool(name="osbp", bufs=2) as osbp,
            tc.tile_pool(name="psum", bufs=6, space="PSUM") as ps,
            tc.tile_pool(name="pss", bufs=1, space="PSUM") as pssp,
            tc.tile_pool(name="dram", bufs=1, space="DRAM") as dram,
        ):
            # ---- constants ----
            posmagic = cpool.tile([128, 1], F32, tag="posmagic")
            nc.vector.memset(posmagic[:], MAGIC)
            negmagic = cpool.tile([128, 1], F32, tag="negmagic")
            nc.vector.memset(negmagic[:], -MAGIC)
            epsb = cpool.tile([128, 1], F32, tag="epsb")
            nc.vector.memset(epsb[:], EPS)
            ones_col = cpool.tile([128, 1], F32, tag="ones_col")
            nc.vector.memset(ones_col[:], 1.0)
            ones_row = cpool.tile([1, 128], F32, tag="ones_row")
            nc.vector.memset(ones_row[:], 1.0)
            alpha_bc = cpool.tile([128, 1], F32, tag="alpha_bc")
            inv_alpha_bc = cpool.tile([128, 1], F32, tag="inv_alpha_bc")
            nw_rep = cpool.tile([128, K], BF16, tag="nw_rep")
            wsum = cpool.tile([128, OT], F32, tag="wsum")

            # resident transposed ternary weights, fp8 (exact for -1/0/1)
            # ot-major layout: [128, OT*KT*128]; o-tile ot owns the contiguous
            # column range [ot*K, (ot+1)*K), kt-subblocks of 128 inside it
            wqT = wres.tile([128, OT * K], FP8, tag="wqT")
            wqT_r = wqT[:].rearrange("p (ot kt f) -> p ot kt f", kt=KT, f=128)

            # replicate norm_weight to all 128 partitions BEFORE the W1 loads
            # claim the SWDGE FIFO: the first hop casts f32->bf16 (SWDGE-only,
            # 16 KB, ~2us), the doubling hops ride the Scalar HWDGE ring so
            # the x-quant chains are not gated on the whole W1 phase.
            nc.gpsimd.dma_start(nw_rep[0:1, :], nw.ap())
            p = 1
            while p < 128:
                nc.scalar.dma_start(nw_rep[p : 2 * p, :], nw_rep[0:p, :])
                p *= 2

            # ---- phase W1: per-shard |w| row sums (queue-exclusive DMAs) ----
            for ot in range(OT):
                wt = big.tile([128, K], F32, tag="bf32a", name=f"wt_{ot}")
                nc.gpsimd.dma_start(wt[:], w[ot * 128 : (ot + 1) * 128, :])
                absw = big.tile([128, K], BF16, tag="s16a", name=f"absw_{ot}", bufs=3)
                nc.scalar.activation(
                    absw[:], wt[:], AFT.Abs, accum_out=wsum[:, ot : ot + 1]
                )

            # ---- x quant chains (fill the collective's latency window) ----
            # x loads ride the Sync HWDGE ring so they are not head-of-line
            # blocked behind the SWDGE weight-load FIFO or the collective.
            sys_ = {}

            def quant_chain(tt):
                xin = big.tile([128, K], F32, tag="bf32a", name=f"xin_{tt}")
                nc.sync.dma_start(xin[:], x[tt * 128 : (tt + 1) * 128, :])

                x2 = big.tile([128, K], BF16, tag="s16a", name=f"x2_{tt}", bufs=3)
                ss = spool.tile([128, 1], F32, tag="ss", name=f"ss_{tt}")
                nc.scalar.activation(x2[:], xin[:], AFT.Square, accum_out=ss[:])

                u = big.tile([128, K], F32, tag="bf32b", name=f"u_{tt}")
                nc.vector.tensor_mul(u[:], xin[:], nw_rep[:])
                graw = spool.tile([128, 1], F32, tag="graw", name=f"graw_{tt}")
                nc.vector.tensor_reduce(
                    graw[:],
                    u[:],
                    axis=mybir.AxisListType.X,
                    op=ALU.max,
                    apply_absolute_value=True,
                )
                g = spool.tile([128, 1], F32, tag="g", name=f"g_{tt}")
                nc.vector.tensor_scalar_max(g[:], graw[:], 1e-10)

                invg = spool.tile([128, 1], F32, tag="invg", name=f"invg_{tt}")
                nc.vector.reciprocal(invg[:], g[:])
                s127 = spool.tile([128, 1], F32, tag="s127", name=f"s127_{tt}")
                nc.vector.tensor_scalar_mul(s127[:], invg[:], 127.0)
                rms = spool.tile([128, 1], F32, tag="rms", name=f"rms_{tt}")
                nc.scalar.activation(
                    rms[:], ss[:], AFT.Sqrt, bias=epsb[:], scale=1.0 / K
                )
                invrms = spool.tile([128, 1], F32, tag="invrms", name=f"invrms_{tt}")
                nc.vector.reciprocal(invrms[:], rms[:])
                gor = spool.tile([128, 1], F32, tag="gor", name=f"gor_{tt}")
                nc.vector.tensor_mul(gor[:], g[:], invrms[:])
                sys_[tt] = gor

                # round(u * 127/g) via magic add/sub; mul+add on ACT, sub on DVE
                q1 = big.tile([128, K], F32, tag="bf32b", name=f"q1_{tt}")
                nc.scalar.activation(
                    q1[:], u[:], AFT.Identity, bias=posmagic[:], scale=s127[:]
                )
                xq = big.tile([128, K], BF16, tag="s16a", name=f"xq_{tt}", bufs=3)
                nc.vector.tensor_scalar_add(xq[:], q1[:], -MAGIC)

                # transpose all KT 128x128 blocks in one DMA-transpose call.
                # DMA_TRANSPOSE occupies the issuing engine for the transfer
                # duration. Prefetch-chain transposes ride the idle Sync
                # engine (before W2 claims it); in-loop ones ride Scalar,
                # which has slack during the matmul loop — this keeps them
                # out of the Sync FIFO behind W2's 16 transposes.
                xqT = big.tile([128, K], BF16, tag="xqT", name=f"xqT_{tt}", bufs=3)
                eng = nc.sync if tt < 3 else nc.scalar
                eng.dma_start(
                    xqT[:].rearrange("p (j f) -> p j f", f=128),
                    xq[:].rearrange("p (j f) -> p j f", f=128),
                    transpose=True,
                )
                return xqT

            xqTs = {}
            NPRE = 3
            for tt in range(min(NPRE, TT)):
                xqTs[tt] = quant_chain(tt)

            # ---- alpha: local reduce -> AllReduce -> matmul broadcast ----
            # the DRAM round-trip DMAs ride the Scalar HWDGE ring: the
            # result readback waits on the collective, and on the SWDGE
            # FIFO it would head-of-line block the W2 weight loads.
            wred = spool.tile([128, 1], F32, tag="wred")
            nc.vector.reduce_sum(wred[:], wsum[:], axis=mybir.AxisListType.X)
            pss = pssp.tile([1, 1], F32, tag="pss", name="pss")
            nc.tensor.matmul(pss[:], wred[:], ones_col[:], start=True, stop=True)
            total_sb = spool.tile([1, 8], F32, tag="total_sb")
            nc.vector.memset(total_sb[:], 0.0)
            nc.vector.tensor_copy(total_sb[:, 0:1], pss[:])

            cc_in = dram.tile([1, 8], F32, tag="cc_in")
            cc_out = dram.tile([1, 8], F32, tag="cc_out")
            nc.scalar.dma_start(cc_in[:], total_sb[:])
            nc.gpsimd.collective_compute(
                "AllReduce",
                ALU.add,
                replica_groups=[list(range(n_cores))],
                ins=[cc_in.opt()],
                outs=[cc_out.opt()],
            )
            gtot = spool.tile([1, 1], F32, tag="gtot")
            nc.scalar.dma_start(gtot[:], cc_out[:, 0:1])
            alpha_s = spool.tile([1, 1], F32, tag="alpha_s")
            nc.vector.tensor_scalar(
                out=alpha_s[:],
                in0=gtot[:],
                scalar1=inv_count,
                scalar2=1e-10,
                op0=ALU.mult,
                op1=ALU.max,
            )
            # broadcast alpha to 128 partitions with one tiny PE matmul
            psb = pssp.tile([128, 1], F32, tag="psb", name="psb")
            nc.tensor.matmul(psb[:], ones_row[:], alpha_s[:], start=True, stop=True)
            nc.scalar.copy(alpha_bc[:], psb[:])
            nc.vector.reciprocal(inv_alpha_bc[:], alpha_bc[:])

            # ---- phase W2: quantize + transpose weights ----
            # round(w/alpha) via magic add/sub stays UNCLIPPED in bf16 (small
            # ints are exact); the clip to [-1,1] fuses into the post-
            # transpose fp8 convert as one dual-op tensor_scalar on DVE.
            # Engine streams are strict in-order, so a single chain paces at
            # the cross-engine ping-pong rate: run even o-tiles entirely on
            # ACT and odd o-tiles entirely on DVE as two independent
            # pipelines (identical fp32 scale*x+bias arithmetic on both).
            for ot in range(OT):
                # split each 2MB load into halves on two DMA paths so the
                # landing latency (~9.6us whole) halves and overlaps compute
                wt2 = big.tile([128, K], F32, tag="bf32a", name=f"wt2_{ot}")
                nc.gpsimd.dma_start(
                    wt2[:, 0 : K // 2], w[ot * 128 : (ot + 1) * 128, 0 : K // 2]
                )
                nc.sync.dma_start(
                    wt2[:, K // 2 : K], w[ot * 128 : (ot + 1) * 128, K // 2 : K]
                )
                # round(w/alpha) via the exact f32 magic (the bf16-magic
                # one-op variant double-rounds: f32 ulp at 192 is 2^-16, so
                # ~1e-5 of weights flip across the +-0.5 thresholds - too
                # many for the error gate). Even o-tiles run on ACT, odd on
                # DVE, as two independent pipelines.
                wdiv = big.tile([128, K], F32, tag="bf32b", name=f"wdiv_{ot}")
                rnd = big.tile([128, K], BF16, tag="s16a", name=f"rnd_{ot}", bufs=3)
                if ot % 2 == 0:
                    nc.scalar.activation(
                        wdiv[:],
                        wt2[:],
                        AFT.Identity,
                        bias=posmagic[:],
                        scale=inv_alpha_bc[:],
                    )
                    nc.scalar.activation(
                        rnd[:], wdiv[:], AFT.Identity, bias=negmagic[:], scale=1.0
                    )
                else:
                    nc.vector.tensor_scalar(
                        out=wdiv[:],
                        in0=wt2[:],
                        scalar1=inv_alpha_bc[:],
                        scalar2=MAGIC,
                        op0=ALU.mult,
                        op1=ALU.add,
                    )
                    nc.vector.tensor_scalar(
                        out=rnd[:],
                        in0=wdiv[:],
                        scalar1=MAGIC,
                        scalar2=-1.0,
                        op0=ALU.subtract,
                        op1=ALU.max,
                    )
                # transpose on the Sync engine (engine-blocking op; Sync is idle)
                rndT = big.tile([128, K], BF16, tag="s16b", name=f"rndT_{ot}")
                nc.sync.dma_start(
                    rndT[:].rearrange("p (j f) -> p j f", f=128),
                    rnd[:].rearrange("p (j f) -> p j f", f=128),
                    transpose=True,
                )
                # fused clip to [-1,1] + convert bf16 -> fp8 on DVE
                # (GPSIMD computes this correctly but ~15x slower: moving it
                # there cost +1ms total - Q7 elementwise is not competitive)
                nc.vector.tensor_scalar(
                    out=wqT[:, ot * K : (ot + 1) * K],
                    in0=rndT[:],
                    scalar1=-1.0,
                    scalar2=1.0,
                    op0=ALU.max,
                    op1=ALU.min,
                )

            # ---- main loop: ob-outer matmuls track the W2 production wave ---
            for tt in range(TT):
                if tt + NPRE < TT:
                    xqTs[tt + NPRE] = quant_chain(tt + NPRE)
                xqT = xqTs.pop(tt)
                gor = sys_.pop(tt)
                sy = spool.tile([128, 1], F32, tag="sy", name=f"sy_{tt}")
                nc.vector.tensor_scalar(
                    out=sy[:],
                    in0=gor[:],
                    scalar1=alpha_bc[:],
                    scalar2=1.0 / 127.0,
                    op0=ALU.mult,
                    op1=ALU.mult,
                )

                # rotated ob start for the first OBN row-tiles: tt consumes
                # weight blocks in the order W2 produces them (tt=0 follows
                # the wave from ob0, tt=1 picks up at ob1, ...), absorbing
                # matmul work into the W2 window while the x-chains stay
                # gated behind W2's buffer ring, so W2 keeps ACT/DVE.
                rot = min(tt, OBN - 1)
                for ob in [(o + rot) % OBN for o in range(OBN)]:
                    psum = ps.tile([128, OBW], F32, tag="ps", name=f"ps_{tt}_{ob}")
                    for kt in range(KT):
                        nc.tensor.matmul(
                            psum[:],
                            xqT[:, kt * 128 : (kt + 1) * 128],
                            wqT_r[:, ob * OTB : (ob + 1) * OTB, kt, :],
                            start=(kt == 0),
                            stop=(kt == KT - 1),
                        )
                    # epilogue on ACT: scale by alpha*gamma/127, store this slice
                    osb = osbp.tile(
                        [128, OBW], F32, tag="osb", name=f"osb_{tt}_{ob}"
                    )
                    nc.scalar.mul(osb[:], psum[:], sy[:])
                    nc.gpsimd.dma_start(
                        y[tt * 128 : (tt + 1) * 128, ob * OBW : (ob + 1) * OBW],
                        osb[:],
                    )

    return nc


_nc_cache = {}


def _get_nc(T, K, O, n_cores):
    key = (T, K, O, n_cores)
    if key not in _nc_cache:
        nc = build(T, K, O, n_cores)
        _split_sync_waits(nc)  # HW-only fixup; CoreSim rejects bare NoOps
        _nc_cache[key] = nc
    return _nc_cache[key]


def kernel(x: np.ndarray, weight: np.ndarray, norm_weight: np.ndarray) -> np.ndarray:
    B, S, K = x.shape
    T = B * S
    Ofull, _ = weight.shape
    O = Ofull // N_CORES

    nc = _get_nc(T, K, O, N_CORES)

    xf = np.ascontiguousarray(x.reshape(T, K).astype(np.float32, copy=False))
    nwf = np.ascontiguousarray(norm_weight.reshape(1, K).astype(np.float32, copy=False))
    in_maps = [
        {
            "x": xf,
            "w": np.ascontiguousarray(weight[i * O : (i + 1) * O]),
            "nw": nwf,
        }
        for i in range(N_CORES)
    ]
    res = run_bass_kernel_spmd(nc, in_maps, list(range(N_CORES))).results
    y = np.concatenate([res[i]["y"] for i in range(N_CORES)], axis=1)
    return y.reshape(B, S, Ofull)


# revision 34
# speedup vs baseline: 1.8868x; 1.0878x over previous
"""BitLinear forward (RMSNorm + absmean ternary weight quant + absmax int8
activation quant + scaled matmul), tensor-parallel over 8 NeuronCores.

Sharding: column-parallel linear — weight rows (out_features) split 8 ways;
x is replicated; alpha (global mean |w|) via a tiny AllReduce; each core
computes y[:, shard] and the host concatenates.

Exactness: quantized activations are integers in [-127, 127] and quantized
weights are in {-1, 0, 1}, so the matmul runs in bf16 (lhsT) x fp8e4 (rhs)
with fp32 PSUM accumulation and is bit-exact (all partial sums < 2^24).

Schedule (v2): W1 weight-scan DMAs get the queue exclusively so the local
|w| sum triggers the AllReduce as early as possible; x-quant chains fill the
collective's latency window; W2 weight-quantize is pipelined across ACT+DVE
with transposes alternating between the two HWDGE rings; the matmul loop
consumes weight column-blocks in production order (ob-outer) so it starts
as soon as the first four o-tiles are quantized.
"""

import numpy as np

import concourse.bass as bass
import concourse.mybir as mybir
import concourse.tile as tile
from concourse.bass_utils import run_bass_kernel_spmd


# The walrus build available here rejects instructions carrying more than one
# attached sync-wait ("Too many sync wait commands"), which Tile emits
# routinely.  Hoist extras onto single-wait NoOps on the same engine —
# engine streams are in-order so wait-then-issue is equivalent.
MAX_ATTACHED_WAITS = 1


def _split_sync_waits(nc, max_waits=MAX_ATTACHED_WAITS):
    nhoisted = 0
    for f in nc.m.functions:
        for blk in f.blocks:
            out = []
            changed = False
            for inst in blk.instructions:
                si = inst.sync_info
                if si is not None and len(si.on_wait) > max_waits:
                    waits = list(si.on_wait)
                    for wt in waits[max_waits:]:
                        out.append(
                            mybir.InstNoOp(
                                name=f"syncsplit-{nc.next_id()}",
                                ins=[],
                                outs=[],
                                engine=inst.engine,
                                sync_info=mybir.SyncInfo(
                                    on_wait=[wt], on_update=[]
                                ),
                                bass_nofuse=True,
                            )
                        )
                        nhoisted += 1
                    inst.sync_info = mybir.SyncInfo(
                        on_wait=waits[:max_waits], on_update=list(si.on_update)
                    )
                    changed = True
                out.append(inst)
            if changed:
                blk.instructions = out
    return nhoisted


F32 = mybir.dt.float32
BF16 = mybir.dt.bfloat16
FP8 = mybir.dt.float8e4

MAGIC = 1.5 * 2.0**23  # add/sub rounds f32 to nearest int (ties to even)
EPS = 1e-6

N_CORES = 8
AFT = mybir.ActivationFunctionType
ALU = mybir.AluOpType


def build(T, K, O, n_cores):
    """One-core SPMD program: x[T,K] f32, w[O,K] f32 shard, nw[1,K] -> y[T,O]."""
    TT, KT, OT = T // 128, K // 128, O // 128
    OBN = max(1, O // 512)  # number of 512-wide output column blocks
    OBW = O // OBN
    assert OBW <= 512
    OTB = OT // OBN  # o-tiles per output block

    nc = bass.Bass(
        "TRN2", target_bir_lowering=False, debug=False, num_devices=n_cores
    )
    x = nc.dram_tensor("x", [T, K], F32, kind="ExternalInput")
    w = nc.dram_tensor("w", [O, K], F32, kind="ExternalInput")
    nw = nc.dram_tensor("nw", [1, K], F32, kind="ExternalInput")
    y = nc.dram_tensor("y", [T, O], F32, kind="ExternalOutput")

    inv_count = 1.0 / (O * n_cores * K)  # power of two for real sizes

    with tile.TileContext(nc) as tc:
        with (
            tc.tile_pool(name="const", bufs=1) as cpool,
            tc.tile_pool(name="wres", bufs=1) as wres,
            tc.tile_pool(name="big", bufs=2) as big,
            tc.tile_pool(name="stat", bufs=6) as spool,
            tc.tile_pool(name="osbp", bufs=4) as osbp,
            tc.tile_pool(name="psum", bufs=7, space="PSUM") as ps,
            tc.tile_pool(name="pss", bufs=1, space="PSUM") as pssp,
            tc.tile_pool(name="dram", bufs=1, space="DRAM") as dram,
        ):
            # ---- constants ----
            posmagic = cpool.tile([128, 1], F32, tag="posmagic")
            nc.vector.memset(posmagic[:], MAGIC)
            negmagic = cpool.tile([128, 1], F32, tag="negmagic")
            nc.vector.memset(negmagic[:], -MAGIC)
            epsb = cpool.tile([128, 1], F32, tag="epsb")
            nc.vector.memset(epsb[:], EPS)
            ones_col = cpool.tile([128, 1], F32, tag="ones_col")
            nc.vector.memset(ones_col[:], 1.0)
            ones_row = cpool.tile([1, 128], F32, tag="ones_row")
            nc.vector.memset(ones_row[:], 1.0)
            alpha_bc = cpool.tile([128, 1], F32, tag="alpha_bc")
            inv_alpha_bc = cpool.tile([128, 1], F32, tag="inv_alpha_bc")
            nw_rep = cpool.tile([128, K], BF16, tag="nw_rep")
            wsum = cpool.tile([128, OT], F32, tag="wsum")

            # resident transposed ternary weights, fp8 (exact for -1/0/1)
            # ot-major layout: [128, OT*KT*128]; o-tile ot owns the contiguous
            # column range [ot*K, (ot+1)*K), kt-subblocks of 128 inside it
            wqT = wres.tile([128, OT * K], FP8, tag="wqT")
            wqT_r = wqT[:].rearrange("p (ot kt f) -> p ot kt f", kt=KT, f=128)

            # replicate norm_weight to all 128 partitions BEFORE the W1 loads
            # claim the SWDGE FIFO: the first hop casts f32->bf16 (SWDGE-only,
            # 16 KB, ~2us), the doubling hops ride the Scalar HWDGE ring so
            # the x-quant chains are not gated on the whole W1 phase.
            nc.gpsimd.dma_start(nw_rep[0:1, :], nw.ap())
            p = 1
            while p < 128:
                nc.scalar.dma_start(nw_rep[p : 2 * p, :], nw_rep[0:p, :])
                p *= 2

            # ---- phase W1: per-shard |w| row sums (queue-exclusive DMAs) ----
            for ot in range(OT):
                wt = big.tile([128, K], F32, tag="bf32a", name=f"wt_{ot}")
                nc.gpsimd.dma_start(wt[:], w[ot * 128 : (ot + 1) * 128, :])
                absw = big.tile([128, K], BF16, tag="s16a", name=f"absw_{ot}")
                nc.scalar.activation(
                    absw[:], wt[:], AFT.Abs, accum_out=wsum[:, ot : ot + 1]
                )

            # ---- x quant chains (fill the collective's latency window) ----
            # x loads ride the Sync HWDGE ring so they are not head-of-line
            # blocked behind the SWDGE weight-load FIFO or the collective.
            sys_ = {}

            def quant_chain(tt):
                xin = big.tile([128, K], F32, tag="bf32a", name=f"xin_{tt}")
                nc.sync.dma_start(xin[:], x[tt * 128 : (tt + 1) * 128, :])

                x2 = big.tile([128, K], BF16, tag="s16a", name=f"x2_{tt}")
                ss = spool.tile([128, 1], F32, tag="ss", name=f"ss_{tt}")
                nc.scalar.activation(x2[:], xin[:], AFT.Square, accum_out=ss[:])

                u = big.tile([128, K], F32, tag="bf32b", name=f"u_{tt}")
                nc.vector.tensor_mul(u[:], xin[:], nw_rep[:])
                graw = spool.tile([128, 1], F32, tag="graw", name=f"graw_{tt}")
                nc.vector.tensor_reduce(
                    graw[:],
                    u[:],
                    axis=mybir.AxisListType.X,
                    op=ALU.max,
                    apply_absolute_value=True,
                )
                g = spool.tile([128, 1], F32, tag="g", name=f"g_{tt}")
                nc.vector.tensor_scalar_max(g[:], graw[:], 1e-10)

                invg = spool.tile([128, 1], F32, tag="invg", name=f"invg_{tt}")
                nc.vector.reciprocal(invg[:], g[:])
                s127 = spool.tile([128, 1], F32, tag="s127", name=f"s127_{tt}")
                nc.vector.tensor_scalar_mul(s127[:], invg[:], 127.0)
                rms = spool.tile([128, 1], F32, tag="rms", name=f"rms_{tt}")
                nc.scalar.activation(
                    rms[:], ss[:], AFT.Sqrt, bias=epsb[:], scale=1.0 / K
                )
                invrms = spool.tile([128, 1], F32, tag="invrms", name=f"invrms_{tt}")
                nc.vector.reciprocal(invrms[:], rms[:])
                gor = spool.tile([128, 1], F32, tag="gor", name=f"gor_{tt}")
                nc.vector.tensor_mul(gor[:], g[:], invrms[:])
                sys_[tt] = gor

                # round(u * 127/g) via magic add/sub; mul+add on ACT, sub on DVE
                q1 = big.tile([128, K], F32, tag="bf32b", name=f"q1_{tt}")
                nc.scalar.activation(
                    q1[:], u[:], AFT.Identity, bias=posmagic[:], scale=s127[:]
                )
                xq = big.tile([128, K], BF16, tag="s16a", name=f"xq_{tt}")
                nc.vector.tensor_scalar_add(xq[:], q1[:], -MAGIC)

                # transpose all KT 128x128 blocks in one DMA-transpose call.
                # DMA_TRANSPOSE occupies the issuing engine for the transfer
                # duration. Prefetch-chain transposes ride the idle Sync
                # engine (before W2 claims it); in-loop ones ride Scalar,
                # which has slack during the matmul loop — this keeps them
                # out of the Sync FIFO behind W2's 16 transposes.
                xqT = big.tile([128, K], BF16, tag="xqT", name=f"xqT_{tt}", bufs=3)
                eng = nc.sync if tt < 3 else nc.scalar
                eng.dma_start(
                    xqT[:].rearrange("p (j f) -> p j f", f=128),
                    xq[:].rearrange("p (j f) -> p j f", f=128),
                    transpose=True,
                )
                return xqT

            xqTs = {}
            NPRE = 3
            for tt in range(min(NPRE, TT)):
                xqTs[tt] = quant_chain(tt)

            # ---- alpha: local reduce -> AllReduce -> matmul broadcast ----
            # the DRAM round-trip DMAs ride the Scalar HWDGE ring: the
            # result readback waits on the collective, and on the SWDGE
            # FIFO it would head-of-line block the W2 weight loads.
            wred = spool.tile([128, 1], F32, tag="wred")
            nc.vector.reduce_sum(wred[:], wsum[:], axis=mybir.AxisListType.X)
            pss = pssp.tile([1, 1], F32, tag="pss", name="pss")
            nc.tensor.matmul(pss[:], wred[:], ones_col[:], start=True, stop=True)
            total_sb = spool.tile([1, 8], F32, tag="total_sb")
            nc.vector.memset(total_sb[:], 0.0)
            nc.vector.tensor_copy(total_sb[:, 0:1], pss[:])

            cc_in = dram.tile([1, 8], F32, tag="cc_in")
            cc_out = dram.tile([1, 8], F32, tag="cc_out")
            nc.scalar.dma_start(cc_in[:], total_sb[:])
            nc.gpsimd.collective_compute(
                "AllReduce",
                ALU.add,
                replica_groups=[list(range(n_cores))],
                ins=[cc_in.opt()],
                outs=[cc_out.opt()],
            )
            gtot = spool.tile([1, 1], F32, tag="gtot")
            nc.scalar.dma_start(gtot[:], cc_out[:, 0:1])
            alpha_s = spool.tile([1, 1], F32, tag="alpha_s")
            nc.vector.tensor_scalar(
                out=alpha_s[:],
                in0=gtot[:],
                scalar1=inv_count,
                scalar2=1e-10,
                op0=ALU.mult,
                op1=ALU.max,
            )
            # broadcast alpha to 128 partitions with one tiny PE matmul
            psb = pssp.tile([128, 1], F32, tag="pss", name="psb")
            nc.tensor.matmul(psb[:], ones_row[:], alpha_s[:], start=True, stop=True)
            nc.scalar.copy(alpha_bc[:], psb[:])
            nc.vector.reciprocal(inv_alpha_bc[:], alpha_bc[:])

            # ---- phase W2: quantize + transpose weights ----
            # round(w/alpha) via magic add/sub stays UNCLIPPED in bf16 (small
            # ints are exact); the clip to [-1,1] fuses into the post-
            # transpose fp8 convert as one dual-op tensor_scalar on DVE.
            # Engine streams are strict in-order, so a single chain paces at
            # the cross-engine ping-pong rate: run even o-tiles entirely on
            # ACT and odd o-tiles entirely on DVE as two independent
            # pipelines (identical fp32 scale*x+bias arithmetic on both).
            for ot in range(OT):
                # split each 2MB load into halves on two DMA paths so the
                # landing latency (~9.6us whole) halves and overlaps compute
                wt2 = big.tile([128, K], F32, tag="bf32a", name=f"wt2_{ot}")
                nc.gpsimd.dma_start(
                    wt2[:, 0 : K // 2], w[ot * 128 : (ot + 1) * 128, 0 : K // 2]
                )
                nc.sync.dma_start(
                    wt2[:, K // 2 : K], w[ot * 128 : (ot + 1) * 128, K // 2 : K]
                )
                # round(w/alpha) via the exact f32 magic (the bf16-magic
                # one-op variant double-rounds: f32 ulp at 192 is 2^-16, so
                # ~1e-5 of weights flip across the +-0.5 thresholds - too
                # many for the error gate). Even o-tiles run on ACT, odd on
                # DVE, as two independent pipelines.
                wdiv = big.tile([128, K], F32, tag="bf32b", name=f"wdiv_{ot}")
                rnd = big.tile([128, K], BF16, tag="s16a", name=f"rnd_{ot}")
                if ot % 2 == 0:
                    nc.scalar.activation(
                        wdiv[:],
                        wt2[:],
                        AFT.Identity,
                        bias=posmagic[:],
                        scale=inv_alpha_bc[:],
                    )
                    nc.scalar.activation(
                        rnd[:], wdiv[:], AFT.Identity, bias=negmagic[:], scale=1.0
                    )
                else:
                    nc.vector.tensor_scalar(
                        out=wdiv[:],
                        in0=wt2[:],
                        scalar1=inv_alpha_bc[:],
                        scalar2=MAGIC,
                        op0=ALU.mult,
                        op1=ALU.add,
                    )
                    nc.vector.tensor_scalar(
                        out=rnd[:],
                        in0=wdiv[:],
                        scalar1=MAGIC,
                        scalar2=-1.0,
                        op0=ALU.subtract,
                        op1=ALU.max,
                    )
                # transpose on the Sync engine (engine-blocking op; Sync is idle)
                rndT = big.tile([128, K], BF16, tag="s16b", name=f"rndT_{ot}")
                nc.sync.dma_start(
                    rndT[:].rearrange("p (j f) -> p j f", f=128),
                    rnd[:].rearrange("p (j f) -> p j f", f=128),
                    transpose=True,
                )
                # fused clip to [-1,1] + convert bf16 -> fp8 on DVE
                # (GPSIMD computes this correctly but ~15x slower: moving it
                # there cost +1ms total - Q7 elementwise is not competitive)
                nc.vector.tensor_scalar(
                    out=wqT[:, ot * K : (ot + 1) * K],
                    in0=rndT[:],
                    scalar1=-1.0,
                    scalar2=1.0,
                    op0=ALU.max,
                    op1=ALU.min,
                )

            # ---- main loop: ob-outer matmuls track the W2 production wave ---
            for tt in range(TT):
                if tt + NPRE < TT:
                    xqTs[tt + NPRE] = quant_chain(tt + NPRE)
                xqT = xqTs.pop(tt)
                gor = sys_.pop(tt)
                sy = spool.tile([128, 1], F32, tag="sy", name=f"sy_{tt}")
                nc.vector.tensor_scalar(
                    out=sy[:],
                    in0=gor[:],
                    scalar1=alpha_bc[:],
                    scalar2=1.0 / 127.0,
                    op0=ALU.mult,
                    op1=ALU.mult,
                )

                # rotated ob start for the first OBN row-tiles: tt consumes
                # weight blocks in the order W2 produces them (tt=0 follows
                # the wave from ob0, tt=1 picks up at ob1, ...), absorbing
                # matmul work into the W2 window while the x-chains stay
                # gated behind W2's buffer ring, so W2 keeps ACT/DVE.
                rot = min(tt, OBN - 1)
                for ob in [(o + rot) % OBN for o in range(OBN)]:
                    psum = ps.tile([128, OBW], F32, tag="ps", name=f"ps_{tt}_{ob}")
                    for kt in range(KT):
                        nc.tensor.matmul(
                            psum[:],
                            xqT[:, kt * 128 : (kt + 1) * 128],
                            wqT_r[:, ob * OTB : (ob + 1) * OTB, kt, :],
                            start=(kt == 0),
                            stop=(kt == KT - 1),
                        )
                    # epilogue on ACT: scale by alpha*gamma/127, store this slice
                    osb = osbp.tile(
                        [128, OBW], F32, tag="osb", name=f"osb_{tt}_{ob}"
                    )
                    nc.scalar.mul(osb[:], psum[:], sy[:])
                    nc.gpsimd.dma_start(
                        y[tt * 128 : (tt + 1) * 128, ob * OBW : (ob + 1) * OBW],
                        osb[:],
                    )

    return nc


_nc_cache = {}


def _get_nc(T, K, O, n_cores):
    key = (T, K, O, n_cores)
    if key not in _nc_cache:
        nc = build(T, K, O, n_cores)
        _split_sync_waits(nc)  # HW-only fixup; CoreSim rejects bare NoOps
        _nc_cache[key] = nc
    return _nc_cache[key]


def kernel(x: np.ndarray, weight: np.ndarray, norm_weight: np.ndarray) -> np.ndarray:
    B, S, K = x.shape
    T = B * S
    Ofull, _ = weight.shape
    O = Ofull // N_CORES

    nc = _get_nc(T, K, O, N_CORES)

    xf = np.ascontiguousarray(x.reshape(T, K).astype(np.float32, copy=False))
    nwf = np.ascontiguousarray(norm_weight.reshape(1, K).astype(np.float32, copy=False))
    in_maps = [
        {
            "x": xf,
            "w": np.ascontiguousarray(weight[i * O : (i + 1) * O]),
            "nw": nwf,
        }
        for i in range(N_CORES)
    ]
    res = run_bass_kernel_spmd(nc, in_maps, list(range(N_CORES))).results
    y = np.concatenate([res[i]["y"] for i in range(N_CORES)], axis=1)
    return y.reshape(B, S, Ofull)
